# revision 51
# baseline (speedup 1.0000x reference)
"""Trainium2 Bass kernel for nn_Net_24429773979977 (dense_mlp).

Computes: 3-layer MLP over [B,T,D]=[2048,128,128] -> f [N,64], row-normalize
u = f/max(||f||,eps), return (||sum u||^2 - sum|u|^2) / (2N).

Strategy (data-parallel over 8 cores, 32768 rows per core):
 - fp8(e4m3) weights/activations for the MLP matmuls.  A normal-mode fp8
   matmul runs at bf16 speed (1 moving row/cycle) but halves x DMA and
   SBUF read bandwidth.  Weights are pre-scaled by powers of 2 (W1*8,
   W2*4, W3*4) to dodge fp8 subnormals; activations carry the scale
   (h1' = 8h1, h2' = 32h2, f' = 128f) and the tail is scale-invariant
   (u = f'/|f'| = f/|f|).  End-to-end rel err ~4.3e-3 (tolerance 2e-2).
 - L3 is a genuinely-2x fp8 DoubleRow matmul: stationary [73, 2, 128]
   with block-diagonal planes ([C|0] plane0, [0|C] plane1, where
   C = [4*W3^T; 8*b3]) so ONE 512-cycle matmul streams both 512-row
   slabs (2 moving rows/cycle, one per weight plane) and emits f for the
   whole pair packed [128, 512] (tile a on partitions 0-63, b on 64-127).
   b3 rides on a constant 16.0-row in h2 (partition 72).
   NOTE: zero-plane DoubleRow for L1/L2 (K<=128) is NOT a win: the PE
   streams both slabs, so padding a dead plane costs the same cycles as
   bf16 while doubling SBUF read traffic (measured 20% SLOWER overall).
   DoubleRow also cannot write at an output-partition offset (col tiling
   is mutually exclusive), hence the merged block-diag form for L3.
 - Pair-level software pipeline (1024 rows per iteration, two 512-row
   tiles packed onto 128 partitions) with multi-stage skew so every
   cross-engine dependency has >= 1 iteration of slack:
     iter p: x-DMA(p+6) | L1(p)+relu1(p) | L2(p-1)+relu2(p-1) |
             L3(p-2)+fev(p-2)+square(p-2) | ones(p-4) [+rsqrt+stt_a at
             group boundary] | stt_b(p-5)
 - relu engines alternate by pair parity (one ACT + one DVE relu per
   iteration; a pair's whole chain stays on one engine).  The engine
   assignment and emission order are a SHARP local optimum: moving any
   slot (fev 3:1 splits, relu2 reassignment, group-wide evictions/stts)
   measured 5-60% slower -- the in-order engine FIFOs convoy.
 - Tail per pair: f evicted PSUM->SBUF bf16 (ACT Copy / DVE cast by
   parity), gpsimd squares it (the only PSUM-free op the Pool engine can
   take), bf16 block-diag ones-matmul broadcasts nsq to both packed
   halves, ACT rsqrt(+eps^2) per 2-pair group, DVE stt u=f*w with
   accum_out giving per-feature row sums.  The kernel is ACT/DVE-bound:
   relu evictions (2048 cols/pair) + tail (1536 cols/pair) at ~1 col/ns
   per engine set the floor; the PE (6 matmuls/pair, warm at 220ns each
   after its HAM activity monitor un-throttles ~3.4us in) and DMA (4MB
   fp8 x per core) both have slack.
 - A dummy rsqrt before the pipeline pulls the ACT table load (the one
   table covering relu/copy/square/abs_reciprocal_sqrt) into the DMA
   wait at startup.
 - PSUM: ps1 [96,1024] + ps2 [72,1024] (bufs=1 each) + ps3 [128,512]
   (bufs=2) + psn [128,1024] (bufs=1) = exactly 8 banks.
 - Host combines per-core partial sums (S) to the final scalar; sum(u*u)
   equals N to fp64 precision since all row norms here are >> eps.
 - ~94.5-95.5us on HW (vs 99.9us bf16 baseline); ~16.5us of that is the
   fixed NEFF preamble (~6.6us sem-clear/preamble-load) + epilogue
   (~10us barrier/queue-drain) shared by any kernel from this toolchain.
"""

import os
from contextlib import ExitStack

import numpy as np

B, T, D = 2048, 128, 128
N = B * T
NCORES = 8
NC_ROWS = N // NCORES          # 32768 rows per core
TILE = 512                     # rows per matmul tile (PSUM bank = 512 fp32)
PAIR_ROWS = 2 * TILE           # two tiles packed into 128 partitions
NPAIRS = NC_ROWS // PAIR_ROWS  # 32
NGROUPS = NPAIRS // 2          # wide-tail groups of 2 pairs
H1, H2, H3 = 96, 72, 64
EPS = 1e-8
ARSQRT_FUNC = "Abs_reciprocal_sqrt"
S1, S2, S3 = 8.0, 4.0, 4.0     # weight pre-scales (powers of 2)


def build_nc():
    import concourse.tile as tile
    from concourse import bacc, mybir

    f32 = mybir.dt.float32
    bf16 = mybir.dt.bfloat16
    f8 = mybir.dt.float8e4

    nc = bacc.Bacc("TRN2", target_bir_lowering=False, debug=False)

    xT = nc.declare_dram_parameter("xT", [D, NC_ROWS], f8, isOutput=False)
    w1t = nc.declare_dram_parameter("w1t", [D, H1], f8, isOutput=False)
    w2t = nc.declare_dram_parameter("w2t", [H1, H2], f8, isOutput=False)
    w3m = nc.declare_dram_parameter("w3m", [H2 + 1, 4 * H3], f8, isOutput=False)
    onesbd = nc.declare_dram_parameter("onesbd", [128, 128], bf16, isOutput=False)
    b1 = nc.declare_dram_parameter("b1", [H1, 1], f32, isOutput=False)
    b2 = nc.declare_dram_parameter("b2", [H2, 1], f32, isOutput=False)
    epsv = nc.declare_dram_parameter("epsv", [128, 1], f32, isOutput=False)

    s_out = nc.declare_dram_parameter("s_out", [128, NPAIRS], f32, isOutput=True)

    add = mybir.AluOpType.add
    mult = mybir.AluOpType.mult
    amax = mybir.AluOpType.max
    DR = mybir.MatmulPerfMode.DoubleRow

    def dr3(ap):
        return ap.rearrange("p (i n) -> p i n", i=2)

    with tile.TileContext(nc) as tc, ExitStack() as ctx:
        consts = ctx.enter_context(tc.tile_pool(name="consts", bufs=1))
        xpool = ctx.enter_context(tc.tile_pool(name="x", bufs=8))
        h1pool = ctx.enter_context(tc.tile_pool(name="h1", bufs=4))
        h2pool = ctx.enter_context(tc.tile_pool(name="h2", bufs=1))
        fpool = ctx.enter_context(tc.tile_pool(name="fsb", bufs=6))
        fsqpool = ctx.enter_context(tc.tile_pool(name="fsq", bufs=5))
        nbpool = ctx.enter_context(tc.tile_pool(name="nb", bufs=3))
        upool = ctx.enter_context(tc.tile_pool(name="u", bufs=1))
        scolpool = ctx.enter_context(tc.tile_pool(name="scol", bufs=1))
        ps1 = ctx.enter_context(tc.tile_pool(name="ps1", bufs=1, space="PSUM"))
        ps2 = ctx.enter_context(tc.tile_pool(name="ps2", bufs=1, space="PSUM"))
        ps3 = ctx.enter_context(tc.tile_pool(name="ps3", bufs=2, space="PSUM"))
        psn = ctx.enter_context(tc.tile_pool(name="psn", bufs=1, space="PSUM"))

        # DMA order tuned for earliest pipeline start: w1+x0 unblock the
        # primer and l1(0); b1 unblocks relu1(0); w2/b2 unblock l2/relu2.
        # x arrives in 4 DMAs (small first tile so l1(0) starts early, then
        # three 10-group slabs) to minimize DMA queue count: the NEFF
        # epilogue drains every queue at ~200ns each.
        w1_sb = consts.tile([D, H1], f8, tag="w1")
        nc.sync.dma_start(out=w1_sb[:], in_=w1t[:])
        b1_sb = consts.tile([H1, 1], f32, tag="b1")
        nc.sync.dma_start(out=b1_sb[:], in_=b1[:])
        b2_sb = consts.tile([H2, 1], f32, tag="b2")
        nc.sync.dma_start(out=b2_sb[:], in_=b2[:])
        eps_sb = consts.tile([128, 1], f32, tag="epsv")
        nc.sync.dma_start(out=eps_sb[:], in_=epsv[:])
        x_first = []
        for _pf in range(3):
            _xt = xpool.tile([D, PAIR_ROWS], f8, tag="xt", name=f"xt_pre{_pf}")
            # issue the prefetch x DMAs from the gpsimd queue: they overlap
            # the const DMAs on Sync instead of serializing behind them
            nc.gpsimd.dma_start(
                out=_xt[:], in_=xT[:, _pf * PAIR_ROWS:(_pf + 1) * PAIR_ROWS])
            x_first.append(_xt)
        w2_sb = consts.tile([H1, H2], f8, tag="w2")
        nc.scalar.dma_start(out=w2_sb[:], in_=w2t[:])
        w3m_sb = consts.tile([H2 + 1, 4 * H3], f8, tag="w3m")
        nc.scalar.dma_start(out=w3m_sb[:], in_=w3m[:])
        ones_sb = consts.tile([128, 128], bf16, tag="ones")
        nc.scalar.dma_start(out=ones_sb[:], in_=onesbd[:])

        scol = scolpool.tile([128, NPAIRS], f32, tag="scol")

        h2_tiles = []
        for i in range(6):
            h2t = h2pool.tile([H2 + 1, PAIR_ROWS], f8, tag=f"h2_{i}")
            nc.gpsimd.memset(h2t[:], 16.0)
            h2_tiles.append(h2t)

        u_scr = upool.tile([128, TILE], bf16, tag="u")

        arsqrt = getattr(mybir.ActivationFunctionType, ARSQRT_FUNC)
        # dummy rsqrt up front: forces the ONE activation table that covers
        # relu+copy+square+abs_reciprocal_sqrt to load during the DMA wait
        # instead of a 1.3us ACT stall at the first real rsqrt mid-pipeline
        tbl_scr = upool.tile([128, 1], f32, tag="tblscr")
        nc.scalar.activation(tbl_scr[:], eps_sb[:], arsqrt, bias=0.0, scale=1.0)
        Relu = mybir.ActivationFunctionType.Relu
        Copy = mybir.ActivationFunctionType.Copy
        Square = mybir.ActivationFunctionType.Square

        xts = {}       # pair -> xt tile
        h1s = {}       # pair -> h1 tile
        p1s = {}       # pair -> ps1 tile
        p2s = {}       # pair -> ps2 tile
        p3s = {}       # pair -> ps3 tile
        fsbs = {}      # pair -> f_sb tile
        fsqs = {}      # pair -> fsq tile
        pns = {}       # group -> wide psn tile
        nbs = {}       # group -> wide nb tile

        def dma_x(p):
            xt = xpool.tile([D, PAIR_ROWS], f8, tag="xt")
            nc.sync.dma_start(
                out=xt[:], in_=xT[:, p * PAIR_ROWS:(p + 1) * PAIR_ROWS])
            xts[p] = xt

        def l1(p):
            p1 = ps1.tile([H1, PAIR_ROWS], f32, tag="ps1")
            xt = xts[p]
            nc.tensor.matmul(p1[:, 0:TILE], w1_sb[:], xt[:, 0:TILE],
                             start=True, stop=True)
            nc.tensor.matmul(p1[:, TILE:PAIR_ROWS], w1_sb[:], xt[:, TILE:PAIR_ROWS],
                             start=True, stop=True)
            p1s[p] = p1

        def relu1(p):
            h1t = h1pool.tile([H1, PAIR_ROWS], f8, tag="h1")
            if p % 2 == 0:
                nc.scalar.activation(h1t[:], p1s[p][:], Relu, bias=b1_sb[:], scale=1.0)
            else:
                nc.vector.tensor_scalar(h1t[:], p1s[p][:], b1_sb[:], 0.0,
                                        op0=add, op1=amax)
            h1s[p] = h1t
            del p1s[p]

        def l2(p):
            p2 = ps2.tile([H2, PAIR_ROWS], f32, tag="ps2")
            h1t = h1s[p]
            nc.tensor.matmul(p2[:, 0:TILE], w2_sb[:], h1t[:, 0:TILE],
                             start=True, stop=True)
            nc.tensor.matmul(p2[:, TILE:PAIR_ROWS], w2_sb[:], h1t[:, TILE:PAIR_ROWS],
                             start=True, stop=True)
            p2s[p] = p2
            del h1s[p]

        def relu2(p):
            h2t = h2_tiles[p % 6]
            if p % 2 == 0:
                nc.scalar.activation(h2t[0:H2, :], p2s[p][:], Relu,
                                     bias=b2_sb[:], scale=1.0)
            else:
                nc.vector.tensor_scalar(h2t[0:H2, :], p2s[p][:], b2_sb[:], 0.0,
                                        op0=add, op1=amax)
            del p2s[p]

        def l3(p):
            # single DoubleRow matmul: stationary [73, 2, 128] block-diag
            # ([C|0] plane 0, [0|C] plane 1) -> f for both tiles packed
            # [128, 512] (tile a on partitions 0-63, tile b on 64-127)
            p3 = ps3.tile([128, TILE], f32, tag="ps3", name="p3t")
            rhs = dr3(h2_tiles[p % 6][:])
            nc.tensor.matmul(p3[:, :], dr3(w3m_sb[:]), rhs,
                             start=True, stop=True, perf_mode=DR)
            p3s[p] = p3

        def fev_sq(p):
            fsq = fsqpool.tile([128, TILE], bf16, tag="fsq")
            if p >= NPAIRS - 2:
                # final group: no later L3 needs ps3, so keep p3 alive,
                # square straight from PSUM and let stt read PSUM too
                nc.scalar.activation(fsq[:], p3s[p][:], Square, bias=0.0, scale=1.0)
                fsqs[p] = fsq
                return
            f_sb = fpool.tile([128, TILE], bf16, tag="fsb")
            if p % 2 == 0:
                nc.scalar.activation(f_sb[:], p3s[p][:], Copy, bias=0.0, scale=1.0)
            else:
                nc.vector.tensor_copy(f_sb[:], p3s[p][:])
            fsbs[p] = f_sb
            del p3s[p]
            nc.gpsimd.tensor_tensor(fsq[:], f_sb[:], f_sb[:], op=mult)
            fsqs[p] = fsq

        def ones_mm(p):
            g, half = p // 2, p % 2
            if half == 0:
                pns[g] = psn.tile([128, PAIR_ROWS], f32, tag="psn", name="pnw")
            pn = pns[g]
            off = half * TILE
            nc.tensor.matmul(pn[:, off:off + TILE], ones_sb[:],
                             fsqs[p][:], start=True, stop=True)
            del fsqs[p]

        def _fsrc(p):
            return p3s[p] if p >= NPAIRS - 2 else fsbs.pop(p)

        def rsqrt_stt_a(g):
            nb = nbpool.tile([128, PAIR_ROWS], bf16, tag="nb")
            if g == NGROUPS - 1:
                nc.scalar.activation(nb[:, 0:TILE], pns[g][:, 0:TILE],
                                     arsqrt, bias=eps_sb[:], scale=1.0)
            else:
                nc.scalar.activation(nb[:], pns[g][:], arsqrt, bias=eps_sb[:], scale=1.0)
                del pns[g]
            nbs[g] = nb
            nc.vector.scalar_tensor_tensor(
                u_scr[:], _fsrc(2 * g)[:], 1.0, nb[:, 0:TILE],
                op0=mult, op1=mult, accum_out=scol[:, 2 * g:2 * g + 1])

        def stt_b(g):
            if g == NGROUPS - 1:
                nc.scalar.activation(nbs[g][:, TILE:PAIR_ROWS],
                                     pns[g][:, TILE:PAIR_ROWS],
                                     arsqrt, bias=eps_sb[:], scale=1.0)
                del pns[g]
            nc.vector.scalar_tensor_tensor(
                u_scr[:], _fsrc(2 * g + 1)[:], 1.0, nbs[g][:, TILE:PAIR_ROWS],
                op0=mult, op1=mult, accum_out=scol[:, 2 * g + 1:2 * g + 2])
            del nbs[g]

        for _pf in range(3):
            xts[_pf] = x_first[_pf]
        dma_x(3)
        dma_x(4)
        dma_x(5)
        LAST = NPAIRS + 6
        for p in range(LAST + 1):
            if p + 6 < NPAIRS:
                dma_x(p + 6)
            if p < NPAIRS:
                l1(p)
                relu1(p)
            if 0 <= p - 1 < NPAIRS:
                l2(p - 1)
                relu2(p - 1)
            if 0 <= p - 2 < NPAIRS:
                l3(p - 2)
                fev_sq(p - 2)
            if 0 <= p - 4 < NPAIRS:
                ones_mm(p - 4)
                if (p - 4) % 2 == 1:
                    rsqrt_stt_a((p - 4) // 2)
            if 0 <= p - 5 < NPAIRS and (p - 5) % 2 == 1:
                stt_b((p - 5) // 2)

        nc.sync.dma_start(out=s_out[:], in_=scol[:])

    nc.compile()
    return nc


def _prep_host_inputs(x, W1, b1, W2, b2, W3, b3):
    import ml_dtypes

    f8 = ml_dtypes.float8_e4m3
    bf = ml_dtypes.bfloat16
    xflat = np.ascontiguousarray(x.reshape(N, D))
    in_maps = []

    w1q = np.ascontiguousarray(S1 * W1.T).astype(f8)
    w2q = np.ascontiguousarray(S2 * W2.T).astype(f8)
    w3c = np.concatenate([S3 * W3.T, (S1 * b3).reshape(1, H3)], axis=0)
    # merged L3 stationary [73, 2*128]: plane0 = [C | 0], plane1 = [0 | C]
    w3mv = np.zeros((H2 + 1, 4 * H3), np.float32)
    w3mv[:, 0:H3] = w3c                       # plane0, out partitions 0-63
    w3mv[:, 3 * H3:4 * H3] = w3c              # plane1, out partitions 64-127
    w3mv = w3mv.astype(f8)
    onesbd = np.zeros((128, 128), np.float32)
    onesbd[:H3, :H3] = 1.0
    onesbd[H3:, H3:] = 1.0
    onesbd = onesbd.astype(bf)
    b1c = np.ascontiguousarray(S1 * b1.reshape(H1, 1), dtype=np.float32)
    b2c = np.ascontiguousarray(S1 * S2 * b2.reshape(H2, 1), dtype=np.float32)
    for c in range(NCORES):
        xT_c = np.ascontiguousarray(
            xflat[c * NC_ROWS:(c + 1) * NC_ROWS].T
        ).astype(f8)
        in_maps.append({
            "xT": xT_c, "w1t": w1q, "w2t": w2q, "w3m": w3mv,
            "onesbd": onesbd, "b1": b1c, "b2": b2c,
            "epsv": np.full((128, 1), EPS * EPS, np.float32),
        })
    return in_maps


def _combine(results):
    """results: list of per-core dicts with s_out [128, NPAIRS]."""
    S = np.zeros(H3, np.float64)
    nrows = 0
    for r in results:
        sc = np.asarray(r["s_out"], np.float64)
        S += sc[:H3].sum(axis=1) + sc[H3:128].sum(axis=1)
        nrows += NC_ROWS
    pair = 0.5 * (S @ S - float(nrows))
    return np.float32(pair / N)


_NC_CACHE = {}


def kernel(x, W1, b1, W2, b2, W3, b3):
    from concourse.bass_utils import run_bass_kernel_spmd

    if "nc" not in _NC_CACHE:
        _NC_CACHE["nc"] = build_nc()
    nc = _NC_CACHE["nc"]
    in_maps = _prep_host_inputs(
        np.asarray(x, np.float32), np.asarray(W1, np.float32),
        np.asarray(b1, np.float32), np.asarray(W2, np.float32),
        np.asarray(b2, np.float32), np.asarray(W3, np.float32),
        np.asarray(b3, np.float32),
    )
    res = run_bass_kernel_spmd(nc, in_maps, list(range(NCORES)))
    return _combine(res.results)


if __name__ == "__main__":
    pass
